# revision 42
# baseline (speedup 1.0000x reference)
"""BEV-pool (segment-sum scatter) Trainium2 kernel for nn_BaseDepthTransform.

Design:
  Host (numpy): replicate the reference geometry -> per-point flat BEV bin id
  (depends only on the small camera matrices, not on x). Sort points by bin.
  Greedily cut the sorted stream into "groups" spanning < W=16 distinct bins.
  Within a group, points of the same bin are packed two-per-lane; 128 lanes
  form a PAIR-TILE holding 256 points as [128, 160]: columns 0:80 are the
  "A" point's features, 80:160 the "B" point's (zeros when a bin has an odd
  count). Both halves share one [128 x 16] one-hot (one lid per lane), so a
  single LDWEIGHTS+MATMUL (N=160 moving columns) reduces 256 points --
  halving the Tensor instruction stream (fewer 256-instruction iram pages =
  fewer instruction-fetch stalls under HBM contention) and halving the
  weight-load overhead.

  Groups hold up to 4 pair-tiles (512 lanes); tile counts are binary-
  decomposed into classes {4,2,1} for a uniform static SPMD schedule across
  the 8 cores. Features ship as fp8 e3m4 (1B; ~1.34e-2 global quant error
  vs the 2e-2 budget; e4m3 measures 2.66e-2 and fails). The one-hot is
  built ON DEVICE by the idle Vector engine (is_equal against an iota
  constant) from a 1-byte-per-lane lid stream embedded in the feats chunk.

  Device (Bass/Tile, SPMD x8): per group, chain pair-matmuls accumulating
  [16, 160] partial sums in PSUM (A|B halves side by side, 3 slots/bank),
  Scalar-engine copy PSUM->SBUF (bf16), GpSimd-triggered DMA out. PSUM
  accumulation chains are interleaved only ACROSS banks (same-bank
  interleaved open chains corrupt).

  Host reassembly: grid[base:base+16] += out[:, gi, :80] + out[:, gi, 80:].
"""
import sys
sys.path.insert(0, '/opt/trn_rl_repo')

import numpy as np
import ml_dtypes

FP8E3 = ml_dtypes.float8_e3m4   # feats + lids + one-hot dtype

# ---- static problem config (mirrors the reference) ----
IH, IW = 256, 704
FH, FW = 32, 88
D = 118
C = 80
C2 = 2 * C
NXg, NYg, NZg = 360, 360, 1
BXc = np.array([-53.85, -53.85, 0.0], np.float32)
DXc = np.array([0.3, 0.3, 20.0], np.float32)
NBINS = NZg * NXg * NYg  # 129600
W = 16                   # bins per group window
MAXLANES = 512           # max lanes per group (4 pair-tiles)
NCORES = 8
CLASSES = (4, 2, 1)      # pair-tiles per chain segment
# groups per DMA chunk, per class (~15.5KB/partition fp8 per chunk;
# class-1 chunks kept tiny so the first chunk lands fast at startup)
CHUNK_GROUPS = {4: 36, 2: 48, 1: 16}
PSUM_SLOTS_PER_BANK = 3  # 3 x 160 f32 = 480 of 512
WAVE = 12                # groups per PSUM wave (2 col groups x 2 banks)
LID_PAD = 32.0           # lid value for padded lanes: not in 0..15, e3m4-exact

_BUILD_CACHE = {}


def _frustum():
    ds = np.arange(1.0, 60.0, 0.5, dtype=np.float32)
    xs = np.linspace(0.0, IW - 1.0, FW, dtype=np.float32)
    ys = np.linspace(0.0, IH - 1.0, FH, dtype=np.float32)
    ds_g = np.broadcast_to(ds[:, None, None], (D, FH, FW))
    xs_g = np.broadcast_to(xs[None, None, :], (D, FH, FW))
    ys_g = np.broadcast_to(ys[None, :, None], (D, FH, FW))
    return np.stack([xs_g, ys_g, ds_g], axis=-1)  # [D,FH,FW,3]


def _get_geometry(c2l_rots, c2l_trans, intrins, post_rots, post_trans,
                  extra_rots, extra_trans):
    fr = _frustum()
    pts = fr[None, None] - post_trans[:, :, None, None, None, :]
    inv_pr = np.linalg.inv(post_rots).astype(np.float32)
    pts = np.einsum('bnij,bndhwj->bndhwi', inv_pr, pts).astype(np.float32)
    pts = np.concatenate([pts[..., :2] * pts[..., 2:3], pts[..., 2:3]], axis=-1)
    combine = np.einsum(
        'bnij,bnjk->bnik', c2l_rots, np.linalg.inv(intrins).astype(np.float32)
    ).astype(np.float32)
    pts = np.einsum('bnij,bndhwj->bndhwi', combine, pts).astype(np.float32)
    pts = pts + c2l_trans[:, :, None, None, None, :]
    pts = np.einsum('bij,bndhwj->bndhwi', extra_rots, pts).astype(np.float32)
    pts = pts + extra_trans[:, None, None, None, None, :]
    return pts  # [B,N,D,FH,FW,3]


def _flat_bins(geom):
    """Per-point flat bin id (int64), -1 for dropped points."""
    coords = ((geom - (BXc - DXc / 2.0)) / DXc).astype(np.int32)
    B = coords.shape[0]
    coords = coords.reshape(B, -1, 3)
    cx, cy, cz = coords[..., 0], coords[..., 1], coords[..., 2]
    kept = (cx >= 0) & (cx < NXg) & (cy >= 0) & (cy < NYg) & (cz >= 0) & (cz < NZg)
    flat = ((cz.astype(np.int64) * NXg + cx) * NYg + cy)
    flat = np.where(kept, flat, -1)
    return flat  # [B, Np]


def _pack_lanes(fk_sorted, pidx_sorted):
    """Greedy group cut (window < W bins, <= MAXLANES lanes) + two-points-
    per-lane packing. Returns per-group dicts with lane arrays."""
    n = len(fk_sorted)
    groups = []
    i = 0
    while i < n:
        hi = np.searchsorted(fk_sorted, fk_sorted[i] + W, side='left')
        j = min(i + MAXLANES * 2, hi, n)
        while True:
            seg = fk_sorted[i:j]
            ub, cnts = np.unique(seg, return_counts=True)
            lanes = int(np.ceil(cnts / 2).sum())
            if lanes <= MAXLANES or j <= i + 1:
                break
            j -= 2 * (lanes - MAXLANES)
            j = max(j, i + 1)
        base = int(fk_sorted[i])
        lv = (fk_sorted[i:j] - base).astype(np.int64)
        pix = pidx_sorted[i:j]
        # rank of each point within its bin run
        run_start = np.r_[0, np.nonzero(np.diff(lv))[0] + 1]
        starts_full = run_start[np.searchsorted(
            run_start, np.arange(len(lv)), side='right') - 1]
        r = np.arange(len(lv)) - starts_full
        lanes_per_bin = np.ceil(cnts / 2).astype(np.int64)
        bin_lane0 = np.r_[0, np.cumsum(lanes_per_bin)[:-1]]
        bin_idx = np.searchsorted(ub, fk_sorted[i:j])
        lane_of_pt = bin_lane0[bin_idx] + r // 2
        nlanes = int(lanes_per_bin.sum())
        lid = np.full(nlanes, int(LID_PAD), np.int64)
        apix = np.full(nlanes, -1, np.int64)
        bpix = np.full(nlanes, -1, np.int64)
        ev = (r % 2 == 0)
        lid[lane_of_pt[ev]] = lv[ev]
        apix[lane_of_pt[ev]] = pix[ev]
        bpix[lane_of_pt[~ev]] = pix[~ev]
        groups.append({"base": base, "lid": lid, "apix": apix, "bpix": bpix})
        i = j
    return groups


def _decompose(groups):
    """Binary-decompose each group's pair-tile count into CLASSES segments:
    (cls, group_ref, lane_start, nlanes, base) in stream order."""
    segs = []
    for g in groups:
        nlanes = len(g["lid"])
        pt = (nlanes + 127) // 128
        s = 0
        for c in CLASSES:
            while pt >= c:
                ln = min(c * 128, nlanes - s)
                segs.append((c, g, s, ln, g["base"]))
                s += ln
                pt -= c
    return segs


def _split_classes(segs):
    """Per class: contiguous split across cores balanced by group count."""
    out = {}
    for c in CLASSES:
        cl = [s for s in segs if s[0] == c]
        G = len(cl)
        per = []
        for ci in range(NCORES):
            lo = (G * ci) // NCORES
            hi = (G * (ci + 1)) // NCORES
            per.append(cl[lo:hi])
        Gmax = max(1, max(len(p) for p in per))
        out[c] = (per, Gmax)
    return out


def _build_core_inputs(class_split, xflat_q):
    """Per-core input dict: per class combined [lids | feats] streams."""
    xpad = np.vstack([xflat_q, np.zeros((1, C), FP8E3)])  # idx -1 -> zeros
    maps = [dict() for _ in range(NCORES)]
    meta = {c: [] for c in CLASSES}
    for c in CLASSES:
        per, Gmax = class_split[c]
        CH = CHUNK_GROUPS[c]
        nch = (Gmax + CH - 1) // CH
        TC = CH * c                  # pair-tiles per (full) chunk
        Tp = nch * TC
        for ci in range(NCORES):
            segs = per[ci]
            feats = np.zeros((Tp, 128, C2), FP8E3)
            lids = np.full((Tp, 128), LID_PAD, FP8E3)
            bases = np.full((Gmax,), -1, np.int64)
            for gi, (_, g, s, ln, base) in enumerate(segs):
                bases[gi] = base
                t0 = gi * c
                nt = (ln + 127) // 128
                for k in range(nt):
                    a, b = s + k * 128, s + min((k + 1) * 128, ln)
                    m = b - a
                    feats[t0 + k, :m, :C] = xpad[g["apix"][a:b]]
                    feats[t0 + k, :m, C:] = xpad[g["bpix"][a:b]]
                    lids[t0 + k, :m] = g["lid"][a:b].astype(FP8E3)
            f = feats.reshape(nch, TC, 128, C2).transpose(0, 2, 1, 3) \
                     .reshape(nch, 128, TC * C2)
            l8 = lids.reshape(nch, TC, 128).transpose(0, 2, 1) \
                     .reshape(nch, 128, TC)
            maps[ci][f"feats{c}"] = np.ascontiguousarray(
                np.concatenate([l8, f], axis=2))
            meta[c].append(bases)
    iota = np.broadcast_to(np.arange(W, dtype=np.float32), (128, W))
    for ci in range(NCORES):
        maps[ci]["iota"] = np.ascontiguousarray(iota.astype(FP8E3))
    return maps, meta


def _build_bass(shape_key):
    """shape_key: tuple of (cls, Gmax) pairs."""
    if shape_key in _BUILD_CACHE:
        return _BUILD_CACHE[shape_key]
    from concourse import bass, mybir, tile, bacc

    nc = bacc.Bacc()
    params = {}
    for c, Gmax in shape_key:
        CH = CHUNK_GROUPS[c]
        nch = (Gmax + CH - 1) // CH
        params[f"feats{c}"] = nc.declare_dram_parameter(
            f"feats{c}", [nch, 128, CH * c * (C2 + 1)], mybir.dt.float8e3,
            isOutput=False)
        params[f"out{c}"] = nc.declare_dram_parameter(
            f"out{c}", [W, Gmax, C2], mybir.dt.bfloat16, isOutput=True)
    params["iota"] = nc.declare_dram_parameter(
        "iota", [128, W], mybir.dt.float8e3, isOutput=False)

    def slot_off(s):
        return (s // PSUM_SLOTS_PER_BANK) * 512 + (s % PSUM_SLOTS_PER_BANK) * C2

    # small classes first so the PE starts on a cheap chunk while the dense
    # class-4 stream is still in flight; a small class last shortens the tail
    chunk_order = []
    for c, Gmax in shape_key:
        CH = CHUNK_GROUPS[c]
        nch = (Gmax + CH - 1) // CH
        for ch in range(nch):
            chunk_order.append((c, Gmax, ch))
    chunk_order.sort(key=lambda t: (t[0] if t[0] != 4 else 3, t[2]))

    with tile.TileContext(nc) as tc:
        with tc.tile_pool(name="fstream", bufs=5) as fpool, \
             tc.tile_pool(name="stage", bufs=6) as spool, \
             tc.tile_pool(name="const", bufs=1) as cpool, \
             tc.tile_pool(name="psum", bufs=4, space="PSUM") as psum_pool:
            iota_t = cpool.tile([128, W], mybir.dt.float8e3, tag="iota")
            nc.sync.dma_start(iota_t[:], params["iota"][:, :])
            for c, Gmax, ch in chunk_order:
                CH = CHUNK_GROUPS[c]
                CHg = min(CH, Gmax - ch * CH)   # groups in this chunk
                T = CHg * c                     # live pair-tiles in chunk
                TC = CH * c
                # chunk dram layout: [lids (TC bytes) | feats (TC*C2)];
                # lids land via a tiny first DMA so the one-hot build
                # overlaps the feats DMA
                lchunk = fpool.tile([128, TC], mybir.dt.float8e3,
                                    tag="lchunk")
                nc.sync.dma_start(lchunk[:], params[f"feats{c}"][ch, :, :TC])
                fchunk = fpool.tile([128, TC * C2], mybir.dt.float8e3,
                                    tag="fchunk")
                nc.sync.dma_start(fchunk[:, :T * C2],
                                  params[f"feats{c}"][ch, :, TC:TC + T * C2])
                ochunk = fpool.tile([128, TC * W], mybir.dt.float8e3,
                                    tag="ochunk")
                # one-hot build: oh[p, t*W+j] = (lids[p,t] == iota[p,j])
                dst = bass.AP(ochunk[:].tensor, ochunk[:].offset,
                              [ochunk[:].ap[0], [W, T], [1, W]])
                src0 = bass.AP(lchunk[:].tensor, lchunk[:].offset,
                               [lchunk[:].ap[0], [1, T], [0, W]])
                src1 = bass.AP(iota_t[:].tensor, iota_t[:].offset,
                               [iota_t[:].ap[0], [0, T], [1, W]])
                nc.vector.tensor_tensor(dst, src0, src1,
                                        mybir.AluOpType.is_equal)
                nwave = (CHg + WAVE - 1) // WAVE
                for wv in range(nwave):
                    g0 = wv * WAVE
                    NW = min(WAVE, CHg - g0)
                    # two PE column groups: wave groups [0,h) accumulate at
                    # PSUM partitions 0:16 (col_grp 0), groups [h,NW) at
                    # 32:48 (col_grp 1, inferred from out.base_partition).
                    # LDWEIGHTS for one col group overlaps the other group's
                    # streaming MM (per-subarray concurrency).
                    h = min(NW, 2 * PSUM_SLOTS_PER_BANK)
                    mega = psum_pool.tile([48, 2 * 512], mybir.dt.float32,
                                          tag="ps")

                    def mm(w, k):
                        ta = (g0 + w) * c + k
                        s = w if w < h else w - h
                        p0 = 0 if w < h else 32
                        off = (s // PSUM_SLOTS_PER_BANK) * 512 \
                            + (s % PSUM_SLOTS_PER_BANK) * C2
                        nc.tensor.matmul(
                            out=mega[p0:p0 + W, off:off + C2],
                            lhsT=ochunk[:, ta * W:(ta + 1) * W],
                            rhs=fchunk[:, ta * C2:(ta + 1) * C2],
                            start=(k == 0), stop=(k == c - 1))

                    if NW == WAVE:
                        # interleave chain pairs across col groups AND banks
                        for j in range(h):
                            wB = h + (j + PSUM_SLOTS_PER_BANK) % h
                            for k in range(c):
                                mm(j, k)
                                mm(wB, k)
                    else:
                        for w in range(NW):   # ragged wave: solo chains
                            for k in range(c):
                                mm(w, k)
                    st = spool.tile([48, 2 * PSUM_SLOTS_PER_BANK * C2],
                                    mybir.dt.bfloat16, tag="st")
                    # per col group: all full banks in one strided ACT op
                    BW = PSUM_SLOTS_PER_BANK * C2          # 480
                    for p0, n0, n1 in ((0, 0, h), (32, h, NW)):
                        ns = n1 - n0
                        if ns <= 0:
                            continue
                        nfull = ns // PSUM_SLOTS_PER_BANK
                        if nfull:
                            sub_m = mega[p0:p0 + W, :]
                            sub_s = st[p0:p0 + W, :]
                            src_ap = bass.AP(
                                sub_m.tensor, sub_m.offset,
                                [sub_m.ap[0], [512, nfull], [1, BW]])
                            dst_ap = bass.AP(
                                sub_s.tensor, sub_s.offset,
                                [sub_s.ap[0], [BW, nfull], [1, BW]])
                            nc.scalar.copy(dst_ap, src_ap)
                        rem = ns - nfull * PSUM_SLOTS_PER_BANK
                        if rem:
                            nc.scalar.copy(
                                st[p0:p0 + W,
                                   nfull * BW:nfull * BW + rem * C2],
                                mega[p0:p0 + W,
                                     nfull * 512:nfull * 512 + rem * C2])
                        nc.gpsimd.dma_start(
                            params[f"out{c}"][:, ch * CH + g0 + n0:
                                              ch * CH + g0 + n1, :],
                            st[p0:p0 + W, :ns * C2])
    nc.finalize()
    _BUILD_CACHE[shape_key] = nc
    return nc


def run_scheduled(x, flat, trace=False, trace_cores=None):
    """Core pipeline given precomputed flat bins; returns (grid, results)."""
    from concourse.bass_utils import run_bass_kernel_spmd

    xflat_q = np.ascontiguousarray(x.reshape(-1, C)).astype(FP8E3)
    kept_idx = np.nonzero(flat >= 0)[0]
    fk = flat[kept_idx]
    order = np.argsort(fk, kind='stable')
    fk_sorted = fk[order]
    pidx_sorted = kept_idx[order]

    groups = _pack_lanes(fk_sorted, pidx_sorted)
    segs = _decompose(groups)
    class_split = _split_classes(segs)
    shape_key = tuple((c, class_split[c][1]) for c in CLASSES)

    maps, meta = _build_core_inputs(class_split, xflat_q)
    nc = _build_bass(shape_key)
    res = run_bass_kernel_spmd(nc, maps, core_ids=list(range(NCORES)),
                               trace=trace, trace_cores=trace_cores)

    grid = np.zeros((NBINS + W, C), np.float32)
    for c in CLASSES:
        for ci in range(NCORES):
            outs = np.asarray(res.results[ci][f"out{c}"],
                              np.float32)          # [W, Gmax, C2]
            bases = meta[c][ci]
            for gi in range(len(bases)):
                base = bases[gi]
                if base >= 0:
                    grid[base:base + W] += outs[:, gi, :C] + outs[:, gi, C:]
    return grid[:NBINS], res


def kernel(x, camera2lidar_rots, camera2lidar_trans, intrins, post_rots,
           post_trans, extra_rots, extra_trans):
    x = np.asarray(x, np.float32)
    B, N = x.shape[0], x.shape[1]
    assert (B, N) == (1, 6) and x.shape[2:] == (D, FH, FW, C), x.shape

    geom = _get_geometry(
        np.asarray(camera2lidar_rots, np.float32),
        np.asarray(camera2lidar_trans, np.float32),
        np.asarray(intrins, np.float32),
        np.asarray(post_rots, np.float32),
        np.asarray(post_trans, np.float32),
        np.asarray(extra_rots, np.float32),
        np.asarray(extra_trans, np.float32),
    )
    flat = _flat_bins(geom)[0]          # [Np]
    grid, _ = run_scheduled(x, flat)
    outp = grid.reshape(NXg, NYg, C).transpose(2, 0, 1)[None]  # [1,C,NX,NY]
    return np.ascontiguousarray(outp)


# revision 43
# speedup vs baseline: 1.0162x; 1.0162x over previous
"""BEV-pool (segment-sum scatter) Trainium2 kernel for nn_BaseDepthTransform.

Design:
  Host (numpy): replicate the reference geometry -> per-point flat BEV bin id
  (depends only on the small camera matrices, not on x). Sort points by bin.
  Greedily cut the sorted stream into "groups" spanning < W=16 distinct bins.
  Within a group, points of the same bin are packed two-per-lane; 128 lanes
  form a PAIR-TILE holding 256 points as [128, 160]: columns 0:80 are the
  "A" point's features, 80:160 the "B" point's (zeros when a bin has an odd
  count). Both halves share one [128 x 16] one-hot (one lid per lane), so a
  single LDWEIGHTS+MATMUL (N=160 moving columns) reduces 256 points --
  halving the Tensor instruction stream (fewer 256-instruction iram pages =
  fewer instruction-fetch stalls under HBM contention) and halving the
  weight-load overhead.

  Groups hold up to 4 pair-tiles (512 lanes); tile counts are binary-
  decomposed into classes {4,2,1} for a uniform static SPMD schedule across
  the 8 cores. Features ship as fp8 e3m4 (1B; ~1.34e-2 global quant error
  vs the 2e-2 budget; e4m3 measures 2.66e-2 and fails). The one-hot is
  built ON DEVICE by the idle Vector engine (is_equal against an iota
  constant) from a 1-byte-per-lane lid stream embedded in the feats chunk.

  Device (Bass/Tile, SPMD x8): per group, chain pair-matmuls accumulating
  [16, 160] partial sums in PSUM (A|B halves side by side, 3 slots/bank),
  Scalar-engine copy PSUM->SBUF (bf16), GpSimd-triggered DMA out. PSUM
  accumulation chains are interleaved only ACROSS banks (same-bank
  interleaved open chains corrupt).

  Host reassembly: grid[base:base+16] += out[:, gi, :80] + out[:, gi, 80:].
"""
import sys
sys.path.insert(0, '/opt/trn_rl_repo')

import numpy as np
import ml_dtypes

FP8E3 = ml_dtypes.float8_e3m4   # feats + lids + one-hot dtype

# ---- static problem config (mirrors the reference) ----
IH, IW = 256, 704
FH, FW = 32, 88
D = 118
C = 80
C2 = 2 * C
NXg, NYg, NZg = 360, 360, 1
BXc = np.array([-53.85, -53.85, 0.0], np.float32)
DXc = np.array([0.3, 0.3, 20.0], np.float32)
NBINS = NZg * NXg * NYg  # 129600
W = 16                   # bins per group window
MAXLANES = 512           # max lanes per group (4 pair-tiles)
NCORES = 8
CLASSES = (4, 2, 1)      # pair-tiles per chain segment
# groups per DMA chunk, per class (~15.5KB/partition fp8 per chunk;
# class-1 chunks kept tiny so the first chunk lands fast at startup)
CHUNK_GROUPS = {4: 24, 2: 48, 1: 16}
PSUM_SLOTS_PER_BANK = 3  # 3 x 160 f32 = 480 of 512
WAVE = 12                # groups per PSUM wave (2 col groups x 2 banks)
LID_PAD = 32.0           # lid value for padded lanes: not in 0..15, e3m4-exact

_BUILD_CACHE = {}


def _frustum():
    ds = np.arange(1.0, 60.0, 0.5, dtype=np.float32)
    xs = np.linspace(0.0, IW - 1.0, FW, dtype=np.float32)
    ys = np.linspace(0.0, IH - 1.0, FH, dtype=np.float32)
    ds_g = np.broadcast_to(ds[:, None, None], (D, FH, FW))
    xs_g = np.broadcast_to(xs[None, None, :], (D, FH, FW))
    ys_g = np.broadcast_to(ys[None, :, None], (D, FH, FW))
    return np.stack([xs_g, ys_g, ds_g], axis=-1)  # [D,FH,FW,3]


def _get_geometry(c2l_rots, c2l_trans, intrins, post_rots, post_trans,
                  extra_rots, extra_trans):
    fr = _frustum()
    pts = fr[None, None] - post_trans[:, :, None, None, None, :]
    inv_pr = np.linalg.inv(post_rots).astype(np.float32)
    pts = np.einsum('bnij,bndhwj->bndhwi', inv_pr, pts).astype(np.float32)
    pts = np.concatenate([pts[..., :2] * pts[..., 2:3], pts[..., 2:3]], axis=-1)
    combine = np.einsum(
        'bnij,bnjk->bnik', c2l_rots, np.linalg.inv(intrins).astype(np.float32)
    ).astype(np.float32)
    pts = np.einsum('bnij,bndhwj->bndhwi', combine, pts).astype(np.float32)
    pts = pts + c2l_trans[:, :, None, None, None, :]
    pts = np.einsum('bij,bndhwj->bndhwi', extra_rots, pts).astype(np.float32)
    pts = pts + extra_trans[:, None, None, None, None, :]
    return pts  # [B,N,D,FH,FW,3]


def _flat_bins(geom):
    """Per-point flat bin id (int64), -1 for dropped points."""
    coords = ((geom - (BXc - DXc / 2.0)) / DXc).astype(np.int32)
    B = coords.shape[0]
    coords = coords.reshape(B, -1, 3)
    cx, cy, cz = coords[..., 0], coords[..., 1], coords[..., 2]
    kept = (cx >= 0) & (cx < NXg) & (cy >= 0) & (cy < NYg) & (cz >= 0) & (cz < NZg)
    flat = ((cz.astype(np.int64) * NXg + cx) * NYg + cy)
    flat = np.where(kept, flat, -1)
    return flat  # [B, Np]


def _pack_lanes(fk_sorted, pidx_sorted):
    """Greedy group cut (window < W bins, <= MAXLANES lanes) + two-points-
    per-lane packing. Returns per-group dicts with lane arrays."""
    n = len(fk_sorted)
    groups = []
    i = 0
    while i < n:
        hi = np.searchsorted(fk_sorted, fk_sorted[i] + W, side='left')
        j = min(i + MAXLANES * 2, hi, n)
        while True:
            seg = fk_sorted[i:j]
            ub, cnts = np.unique(seg, return_counts=True)
            lanes = int(np.ceil(cnts / 2).sum())
            if lanes <= MAXLANES or j <= i + 1:
                break
            j -= 2 * (lanes - MAXLANES)
            j = max(j, i + 1)
        base = int(fk_sorted[i])
        lv = (fk_sorted[i:j] - base).astype(np.int64)
        pix = pidx_sorted[i:j]
        # rank of each point within its bin run
        run_start = np.r_[0, np.nonzero(np.diff(lv))[0] + 1]
        starts_full = run_start[np.searchsorted(
            run_start, np.arange(len(lv)), side='right') - 1]
        r = np.arange(len(lv)) - starts_full
        lanes_per_bin = np.ceil(cnts / 2).astype(np.int64)
        bin_lane0 = np.r_[0, np.cumsum(lanes_per_bin)[:-1]]
        bin_idx = np.searchsorted(ub, fk_sorted[i:j])
        lane_of_pt = bin_lane0[bin_idx] + r // 2
        nlanes = int(lanes_per_bin.sum())
        lid = np.full(nlanes, int(LID_PAD), np.int64)
        apix = np.full(nlanes, -1, np.int64)
        bpix = np.full(nlanes, -1, np.int64)
        ev = (r % 2 == 0)
        lid[lane_of_pt[ev]] = lv[ev]
        apix[lane_of_pt[ev]] = pix[ev]
        bpix[lane_of_pt[~ev]] = pix[~ev]
        groups.append({"base": base, "lid": lid, "apix": apix, "bpix": bpix})
        i = j
    return groups


def _decompose(groups):
    """Binary-decompose each group's pair-tile count into CLASSES segments:
    (cls, group_ref, lane_start, nlanes, base) in stream order."""
    segs = []
    for g in groups:
        nlanes = len(g["lid"])
        pt = (nlanes + 127) // 128
        s = 0
        for c in CLASSES:
            while pt >= c:
                ln = min(c * 128, nlanes - s)
                segs.append((c, g, s, ln, g["base"]))
                s += ln
                pt -= c
    return segs


def _split_classes(segs):
    """Per class: contiguous split across cores balanced by group count."""
    out = {}
    for c in CLASSES:
        cl = [s for s in segs if s[0] == c]
        G = len(cl)
        per = []
        for ci in range(NCORES):
            lo = (G * ci) // NCORES
            hi = (G * (ci + 1)) // NCORES
            per.append(cl[lo:hi])
        Gmax = max(1, max(len(p) for p in per))
        out[c] = (per, Gmax)
    return out


def _build_core_inputs(class_split, xflat_q):
    """Per-core input dict: per class combined [lids | feats] streams."""
    xpad = np.vstack([xflat_q, np.zeros((1, C), FP8E3)])  # idx -1 -> zeros
    maps = [dict() for _ in range(NCORES)]
    meta = {c: [] for c in CLASSES}
    for c in CLASSES:
        per, Gmax = class_split[c]
        CH = CHUNK_GROUPS[c]
        nch = (Gmax + CH - 1) // CH
        TC = CH * c                  # pair-tiles per (full) chunk
        Tp = nch * TC
        for ci in range(NCORES):
            segs = per[ci]
            feats = np.zeros((Tp, 128, C2), FP8E3)
            lids = np.full((Tp, 128), LID_PAD, FP8E3)
            bases = np.full((Gmax,), -1, np.int64)
            for gi, (_, g, s, ln, base) in enumerate(segs):
                bases[gi] = base
                t0 = gi * c
                nt = (ln + 127) // 128
                for k in range(nt):
                    a, b = s + k * 128, s + min((k + 1) * 128, ln)
                    m = b - a
                    feats[t0 + k, :m, :C] = xpad[g["apix"][a:b]]
                    feats[t0 + k, :m, C:] = xpad[g["bpix"][a:b]]
                    lids[t0 + k, :m] = g["lid"][a:b].astype(FP8E3)
            f = feats.reshape(nch, TC, 128, C2).transpose(0, 2, 1, 3) \
                     .reshape(nch, 128, TC * C2)
            l8 = lids.reshape(nch, TC, 128).transpose(0, 2, 1) \
                     .reshape(nch, 128, TC)
            maps[ci][f"feats{c}"] = np.ascontiguousarray(
                np.concatenate([l8, f], axis=2))
            meta[c].append(bases)
    iota = np.broadcast_to(np.arange(W, dtype=np.float32), (128, W))
    for ci in range(NCORES):
        maps[ci]["iota"] = np.ascontiguousarray(iota.astype(FP8E3))
    return maps, meta


def _build_bass(shape_key):
    """shape_key: tuple of (cls, Gmax) pairs."""
    if shape_key in _BUILD_CACHE:
        return _BUILD_CACHE[shape_key]
    from concourse import bass, mybir, tile, bacc

    nc = bacc.Bacc()
    params = {}
    for c, Gmax in shape_key:
        CH = CHUNK_GROUPS[c]
        nch = (Gmax + CH - 1) // CH
        params[f"feats{c}"] = nc.declare_dram_parameter(
            f"feats{c}", [nch, 128, CH * c * (C2 + 1)], mybir.dt.float8e3,
            isOutput=False)
        params[f"out{c}"] = nc.declare_dram_parameter(
            f"out{c}", [W, Gmax, C2], mybir.dt.bfloat16, isOutput=True)
    params["iota"] = nc.declare_dram_parameter(
        "iota", [128, W], mybir.dt.float8e3, isOutput=False)

    def slot_off(s):
        return (s // PSUM_SLOTS_PER_BANK) * 512 + (s % PSUM_SLOTS_PER_BANK) * C2

    # small classes first so the PE starts on a cheap chunk while the dense
    # class-4 stream is still in flight; a small class last shortens the tail
    chunk_order = []
    for c, Gmax in shape_key:
        CH = CHUNK_GROUPS[c]
        nch = (Gmax + CH - 1) // CH
        for ch in range(nch):
            chunk_order.append((c, Gmax, ch))
    chunk_order.sort(key=lambda t: (t[0] if t[0] != 4 else 3, t[2]))

    with tile.TileContext(nc) as tc:
        with tc.tile_pool(name="fstream", bufs=5) as fpool, \
             tc.tile_pool(name="stage", bufs=6) as spool, \
             tc.tile_pool(name="const", bufs=1) as cpool, \
             tc.tile_pool(name="psum", bufs=4, space="PSUM") as psum_pool:
            iota_t = cpool.tile([128, W], mybir.dt.float8e3, tag="iota")
            nc.sync.dma_start(iota_t[:], params["iota"][:, :])
            for c, Gmax, ch in chunk_order:
                CH = CHUNK_GROUPS[c]
                CHg = min(CH, Gmax - ch * CH)   # groups in this chunk
                T = CHg * c                     # live pair-tiles in chunk
                TC = CH * c
                # chunk dram layout: [lids (TC bytes) | feats (TC*C2)];
                # lids land via a tiny first DMA so the one-hot build
                # overlaps the feats DMA
                lchunk = fpool.tile([128, TC], mybir.dt.float8e3,
                                    tag="lchunk")
                nc.sync.dma_start(lchunk[:], params[f"feats{c}"][ch, :, :TC])
                fchunk = fpool.tile([128, TC * C2], mybir.dt.float8e3,
                                    tag="fchunk")
                nc.sync.dma_start(fchunk[:, :T * C2],
                                  params[f"feats{c}"][ch, :, TC:TC + T * C2])
                ochunk = fpool.tile([128, TC * W], mybir.dt.float8e3,
                                    tag="ochunk")
                # one-hot build: oh[p, t*W+j] = (lids[p,t] == iota[p,j])
                dst = bass.AP(ochunk[:].tensor, ochunk[:].offset,
                              [ochunk[:].ap[0], [W, T], [1, W]])
                src0 = bass.AP(lchunk[:].tensor, lchunk[:].offset,
                               [lchunk[:].ap[0], [1, T], [0, W]])
                src1 = bass.AP(iota_t[:].tensor, iota_t[:].offset,
                               [iota_t[:].ap[0], [0, T], [1, W]])
                nc.vector.tensor_tensor(dst, src0, src1,
                                        mybir.AluOpType.is_equal)
                nwave = (CHg + WAVE - 1) // WAVE
                for wv in range(nwave):
                    g0 = wv * WAVE
                    NW = min(WAVE, CHg - g0)
                    # two PE column groups: wave groups [0,h) accumulate at
                    # PSUM partitions 0:16 (col_grp 0), groups [h,NW) at
                    # 32:48 (col_grp 1, inferred from out.base_partition).
                    # LDWEIGHTS for one col group overlaps the other group's
                    # streaming MM (per-subarray concurrency).
                    h = min(NW, 2 * PSUM_SLOTS_PER_BANK)
                    mega = psum_pool.tile([48, 2 * 512], mybir.dt.float32,
                                          tag="ps")

                    def mm(w, k):
                        ta = (g0 + w) * c + k
                        s = w if w < h else w - h
                        p0 = 0 if w < h else 32
                        off = (s // PSUM_SLOTS_PER_BANK) * 512 \
                            + (s % PSUM_SLOTS_PER_BANK) * C2
                        nc.tensor.matmul(
                            out=mega[p0:p0 + W, off:off + C2],
                            lhsT=ochunk[:, ta * W:(ta + 1) * W],
                            rhs=fchunk[:, ta * C2:(ta + 1) * C2],
                            start=(k == 0), stop=(k == c - 1))

                    if NW == WAVE:
                        # interleave chain pairs across col groups AND banks
                        for j in range(h):
                            wB = h + (j + PSUM_SLOTS_PER_BANK) % h
                            for k in range(c):
                                mm(j, k)
                                mm(wB, k)
                    else:
                        for w in range(NW):   # ragged wave: solo chains
                            for k in range(c):
                                mm(w, k)
                    st = spool.tile([48, 2 * PSUM_SLOTS_PER_BANK * C2],
                                    mybir.dt.bfloat16, tag="st")
                    # per col group: all full banks in one strided ACT op
                    BW = PSUM_SLOTS_PER_BANK * C2          # 480
                    for p0, n0, n1 in ((0, 0, h), (32, h, NW)):
                        ns = n1 - n0
                        if ns <= 0:
                            continue
                        nfull = ns // PSUM_SLOTS_PER_BANK
                        if nfull:
                            sub_m = mega[p0:p0 + W, :]
                            sub_s = st[p0:p0 + W, :]
                            src_ap = bass.AP(
                                sub_m.tensor, sub_m.offset,
                                [sub_m.ap[0], [512, nfull], [1, BW]])
                            dst_ap = bass.AP(
                                sub_s.tensor, sub_s.offset,
                                [sub_s.ap[0], [BW, nfull], [1, BW]])
                            nc.scalar.copy(dst_ap, src_ap)
                        rem = ns - nfull * PSUM_SLOTS_PER_BANK
                        if rem:
                            nc.scalar.copy(
                                st[p0:p0 + W,
                                   nfull * BW:nfull * BW + rem * C2],
                                mega[p0:p0 + W,
                                     nfull * 512:nfull * 512 + rem * C2])
                        nc.gpsimd.dma_start(
                            params[f"out{c}"][:, ch * CH + g0 + n0:
                                              ch * CH + g0 + n1, :],
                            st[p0:p0 + W, :ns * C2])
    nc.finalize()
    _BUILD_CACHE[shape_key] = nc
    return nc


def run_scheduled(x, flat, trace=False, trace_cores=None):
    """Core pipeline given precomputed flat bins; returns (grid, results)."""
    from concourse.bass_utils import run_bass_kernel_spmd

    xflat_q = np.ascontiguousarray(x.reshape(-1, C)).astype(FP8E3)
    kept_idx = np.nonzero(flat >= 0)[0]
    fk = flat[kept_idx]
    order = np.argsort(fk, kind='stable')
    fk_sorted = fk[order]
    pidx_sorted = kept_idx[order]

    groups = _pack_lanes(fk_sorted, pidx_sorted)
    segs = _decompose(groups)
    class_split = _split_classes(segs)
    shape_key = tuple((c, class_split[c][1]) for c in CLASSES)

    maps, meta = _build_core_inputs(class_split, xflat_q)
    nc = _build_bass(shape_key)
    res = run_bass_kernel_spmd(nc, maps, core_ids=list(range(NCORES)),
                               trace=trace, trace_cores=trace_cores)

    grid = np.zeros((NBINS + W, C), np.float32)
    for c in CLASSES:
        for ci in range(NCORES):
            outs = np.asarray(res.results[ci][f"out{c}"],
                              np.float32)          # [W, Gmax, C2]
            bases = meta[c][ci]
            for gi in range(len(bases)):
                base = bases[gi]
                if base >= 0:
                    grid[base:base + W] += outs[:, gi, :C] + outs[:, gi, C:]
    return grid[:NBINS], res


def kernel(x, camera2lidar_rots, camera2lidar_trans, intrins, post_rots,
           post_trans, extra_rots, extra_trans):
    x = np.asarray(x, np.float32)
    B, N = x.shape[0], x.shape[1]
    assert (B, N) == (1, 6) and x.shape[2:] == (D, FH, FW, C), x.shape

    geom = _get_geometry(
        np.asarray(camera2lidar_rots, np.float32),
        np.asarray(camera2lidar_trans, np.float32),
        np.asarray(intrins, np.float32),
        np.asarray(post_rots, np.float32),
        np.asarray(post_trans, np.float32),
        np.asarray(extra_rots, np.float32),
        np.asarray(extra_trans, np.float32),
    )
    flat = _flat_bins(geom)[0]          # [Np]
    grid, _ = run_scheduled(x, flat)
    outp = grid.reshape(NXg, NYg, C).transpose(2, 0, 1)[None]  # [1,C,NX,NY]
    return np.ascontiguousarray(outp)
